# revision 12
# baseline (speedup 1.0000x reference)
"""Detection-loss kernel for Trainium2 (8 NeuronCores, data-parallel over batch).

Reference computes: scatter 64 targets/image into a [B,C,H,W] map + mask,
then masked SmoothL1(preds, map).sum() / num_objects.

Key observation: the mask is nonzero at <= B*T positions, so the loss only
depends on preds at those positions.  Instead of streaming the 143MB preds
tensor, each core *gathers* preds at its images' (gy,gx) cells via indirect
DMA (1792 elements/core), resolves duplicate-cell collisions with
last-writer-wins (matching jax scatter semantics), and reduces two partial
scalars.  Host combines the 8 partial pairs.

Layout per core (4 images, 2 groups of 128 targets on partitions):
  partition p in [0,128), group g in {0,1}:
    image j = g*2 + p//64 (local), target t = p%64, channel c in [0,7)
    tvals[p, g*7+c]  = targets[j, t, c]
    cbase[p, g*7+c]  = j*C*H*W + c*H*W   (flat element base, f32-exact)
  flat gather offset = cbase + (gy*400 + gx), gy/gx = floor(coord * 5.0)
"""

import numpy as np

B, C, H, W = 32, 7, 400, 400
T = 64
NCORES = 8
BLOC = B // NCORES          # 4 images per core
HW = H * W                  # 160000
CHW = C * HW                # 1120000
NELEM = BLOC * CHW          # 4480000 elements per core
NG = BLOC * T // 128        # 2 groups of 128 targets
P = 128
GC = NG * C                 # 14 columns

_cached = {}
TRACE = False  # set True (e.g. from test.py) to capture an NTFF profile


def _build_nc():
    import concourse.bacc as bacc
    import concourse.bass as bass
    import concourse.tile as tile
    import concourse.mybir as mybir

    f32 = mybir.dt.float32
    i32 = mybir.dt.int32
    AF = mybir.ActivationFunctionType
    OP = mybir.AluOpType
    AX = mybir.AxisListType

    nc = bacc.Bacc(
        "TRN2",
        target_bir_lowering=False,
        debug=False,
        enable_asserts=True,
        num_devices=NCORES,
    )

    preds_flat = nc.dram_tensor("preds_flat", [NELEM, 1], f32, kind="ExternalInput")
    tvals = nc.dram_tensor("tvals", [P, GC], f32, kind="ExternalInput")
    t01 = nc.dram_tensor("t01", [P, 2 * NG], f32, kind="ExternalInput")
    cbase = nc.dram_tensor("cbase", [P, GC], f32, kind="ExternalInput")
    ut = nc.dram_tensor("ut", [P, P], f32, kind="ExternalInput")
    id128 = nc.dram_tensor("id128", [P, P], f32, kind="ExternalInput")
    onehalf = nc.dram_tensor("onehalf", [2, 1], f32, kind="ExternalInput")
    out_d = nc.dram_tensor("out", [2, 1], f32, kind="ExternalOutput")

    with tile.TileContext(nc) as tc:
        with (
            tc.tile_pool(name="sbuf", bufs=1) as sb,
            tc.tile_pool(name="psum", bufs=1, space="PSUM") as pp,
            tc.tile_pool(name="dram", bufs=1, space="DRAM") as dp,
        ):
            tv = sb.tile([P, GC], f32)
            nc.sync.dma_start(tv[:], tvals[:, :])
            txy = sb.tile([P, 2 * NG], f32)
            nc.sync.dma_start(txy[:], t01[:, :])
            cb = sb.tile([P, GC], f32)
            nc.sync.dma_start(cb[:], cbase[:, :])
            utt = sb.tile([P, P], f32)
            nc.sync.dma_start(utt[:], ut[:, :])
            idt = sb.tile([P, P], f32)
            nc.sync.dma_start(idt[:], id128[:, :])
            oht = sb.tile([2, 1], f32)
            nc.sync.dma_start(oht[:], onehalf[:, :])

            # grid coords: fl = floor(coord * 5.0).  HW-safe floor: round-trip
            # through int32 (any rounding mode), then subtract 1 where the
            # round-tripped value exceeds x.
            a = sb.tile([P, 2 * NG], f32)
            nc.vector.tensor_scalar_mul(a[:], txy[:], 5.0)
            ci = sb.tile([P, 2 * NG], i32)
            nc.vector.tensor_copy(ci[:], a[:])
            cf = sb.tile([P, 2 * NG], f32)
            nc.vector.tensor_copy(cf[:], ci[:])
            corr = sb.tile([P, 2 * NG], f32)
            nc.vector.tensor_tensor(corr[:], cf[:], a[:], OP.is_gt)
            fl = sb.tile([P, 2 * NG], f32)
            nc.vector.tensor_sub(fl[:], cf[:], corr[:])
            # npos = gy*400 + gx   (cols 0:NG of fl are gx, NG:2NG are gy)
            gy400 = sb.tile([P, NG], f32)
            nc.vector.tensor_scalar_mul(gy400[:], fl[:, NG : 2 * NG], float(W))
            npos = sb.tile([P, NG], f32)
            nc.vector.tensor_add(npos[:], gy400[:], fl[:, 0:NG])

            # flat element offsets = cbase + npos (per group), exact ints < 2^24
            offs_f = sb.tile([P, GC], f32)
            for g in range(NG):
                nc.vector.tensor_scalar(
                    offs_f[:, g * C : (g + 1) * C],
                    cb[:, g * C : (g + 1) * C],
                    npos[:, g : g + 1],
                    None,
                    OP.add,
                )
            offs_i = sb.tile([P, GC], i32)
            nc.vector.tensor_copy(offs_i[:], offs_f[:])

            # gather preds at the 1792 scattered cells.  HW indirect DMA
            # consumes ONE offset per partition row (contiguous run per row),
            # so issue one [P,1] gather per (group, channel) column.
            gat = sb.tile([P, GC], f32)
            for col in range(GC):
                nc.gpsimd.indirect_dma_start(
                    out=gat[:, col : col + 1],
                    out_offset=None,
                    in_=preds_flat[:, :],
                    in_offset=bass.IndirectOffsetOnAxis(
                        ap=offs_i[:, col : col + 1], axis=0
                    ),
                )

            # 2*smoothl1(d) = min(|d|,1) * (|d| + relu(|d|-1))
            d = sb.tile([P, GC], f32)
            nc.vector.tensor_sub(d[:], gat[:], tv[:])
            nd = sb.tile([P, GC], f32)
            nc.vector.tensor_scalar_mul(nd[:], d[:], -1.0)
            ad = sb.tile([P, GC], f32)
            nc.vector.tensor_tensor(ad[:], d[:], nd[:], OP.max)
            mn = sb.tile([P, GC], f32)
            nc.vector.tensor_scalar_min(mn[:], ad[:], 1.0)
            r = sb.tile([P, GC], f32)
            nc.vector.tensor_scalar(r[:], ad[:], 1.0, 0.0, OP.subtract, OP.max)
            s = sb.tile([P, GC], f32)
            nc.vector.tensor_add(s[:], ad[:], r[:])
            le = sb.tile([P, GC], f32)
            nc.vector.tensor_mul(le[:], mn[:], s[:])

            # last-writer-wins winner mask per group; rhs cols = (loss_row, win)
            rhs = sb.tile([P, 2], f32)
            win2 = sb.tile([P, NG], f32)
            lw2 = sb.tile([P, NG], f32)
            for g in range(NG):
                posb = offs_f[:, g * C : g * C + 1].to_broadcast([P, P])
                pT_ps = pp.tile([P, P], f32, tag=f"tps{g}")
                nc.tensor.transpose(pT_ps[:], posb, idt[:])
                pT = sb.tile([P, P], f32, tag=f"pT{g}")
                nc.vector.tensor_copy(pT[:], pT_ps[:])
                eq = sb.tile([P, P], f32, tag=f"eq{g}")
                nc.vector.tensor_tensor(eq[:], posb, pT[:], OP.is_equal)
                msk = sb.tile([P, P], f32, tag=f"msk{g}")
                nc.vector.tensor_mul(msk[:], eq[:], utt[:])
                coll = sb.tile([P, 1], f32, tag=f"coll{g}")
                nc.vector.reduce_max(coll[:], msk[:], axis=AX.X)
                nc.vector.tensor_scalar(
                    win2[:, g : g + 1], coll[:], -1.0, 1.0, OP.mult, OP.add
                )
                lrow = sb.tile([P, 1], f32, tag=f"lrow{g}")
                nc.vector.reduce_sum(lrow[:], le[:, g * C : (g + 1) * C], axis=AX.X)
                nc.vector.tensor_mul(
                    lw2[:, g : g + 1], lrow[:], win2[:, g : g + 1]
                )
            nc.vector.tensor_add(rhs[:, 0:1], lw2[:, 0:1], lw2[:, 1:2])
            nc.vector.tensor_add(rhs[:, 1:2], win2[:, 0:1], win2[:, 1:2])

            # exact partition reduction: bounce [P,2] through DRAM, read back
            # transposed as [2,P], reduce on DVE (PE fp32 matmul is fp32r on
            # HW and loses precision).  Scale loss row by 0.5 at the end.
            scratch = dp.tile([P, 2], f32)
            nc.sync.dma_start(scratch[:], rhs[:])
            tr = sb.tile([2, P], f32)
            nc.sync.dma_start(tr[:], scratch[:].rearrange("p c -> c p"))
            red = sb.tile([2, 1], f32)
            nc.vector.reduce_sum(red[:], tr[:], axis=AX.X)
            outt = sb.tile([2, 1], f32)
            nc.vector.tensor_tensor(outt[:], red[:], oht[:], OP.mult)
            nc.sync.dma_start(out_d[:, :], outt[:])

    nc.compile()
    return nc


def _get_nc():
    if "nc" not in _cached:
        _cached["nc"] = _build_nc()
    return _cached["nc"]


def _make_in_maps(preds, targets):
    jj = (np.arange(P) // 64)[:, None]
    gg = (np.arange(GC) // C)[None, :]
    cc = (np.arange(GC) % C)[None, :]
    cbase = ((gg * 2 + jj) * CHW + cc * HW).astype(np.float32)
    ut = np.triu(np.ones((P, P), np.float32), k=1)
    id128 = np.eye(P, dtype=np.float32)
    onehalf = np.array([[0.5], [1.0]], np.float32)

    in_maps = []
    for k in range(NCORES):
        pshard = np.ascontiguousarray(preds[k * BLOC : (k + 1) * BLOC]).reshape(
            NELEM, 1
        )
        tshard = targets[k * BLOC : (k + 1) * BLOC]  # [4, 64, 7]
        # tvals[p, g*7+c] = tshard[g*2 + p//64, p%64, c]
        tvals = np.ascontiguousarray(
            tshard.reshape(NG, 2, T, C).transpose(1, 2, 0, 3).reshape(P, GC)
        )
        # t01 cols: [x_g0, x_g1, y_g0, y_g1]
        t01 = np.ascontiguousarray(
            np.stack(
                [tvals[:, 0], tvals[:, C], tvals[:, 1], tvals[:, C + 1]], axis=1
            )
        )
        in_maps.append(
            {
                "preds_flat": pshard,
                "tvals": tvals,
                "t01": t01,
                "cbase": cbase,
                "ut": ut,
                "id128": id128,
                "onehalf": onehalf,
            }
        )
    return in_maps


def kernel(preds, targets):
    from concourse.bass_utils import run_bass_kernel_spmd

    preds = np.ascontiguousarray(np.asarray(preds), dtype=np.float32)
    targets = np.ascontiguousarray(np.asarray(targets), dtype=np.float32)
    assert preds.shape == (B, C, H, W) and targets.shape == (B, T, C)

    nc = _get_nc()
    in_maps = _make_in_maps(preds, targets)
    res = run_bass_kernel_spmd(nc, in_maps, list(range(NCORES)), trace=TRACE)
    _cached["last_results"] = res

    lsum = np.float32(0.0)
    nsum = np.float32(0.0)
    for k in range(NCORES):
        part = res.results[k]["out"].reshape(2)
        lsum = np.float32(lsum + np.float32(part[0]))
        nsum = np.float32(nsum + np.float32(part[1]))
    loss = np.float32(lsum / np.float32(nsum + np.float32(1e-6)))
    return loss, nsum


# revision 18
# speedup vs baseline: 1.4904x; 1.4904x over previous
"""Detection-loss kernel for Trainium2 (8 NeuronCores, data-parallel over batch).

Reference computes: scatter 64 targets/image into a [B,C,H,W] map + mask,
then masked SmoothL1(preds, map).sum() / num_objects.

Key observation: the mask is nonzero at <= B*T positions, so the loss only
depends on preds at those positions.  Instead of streaming the 143MB preds
tensor, each core *gathers* preds at its images' (gy,gx) cells via indirect
DMA (1792 elements/core), resolves duplicate-cell collisions with
last-writer-wins (matching jax scatter semantics), and reduces two partial
scalars.  Host combines the 8 partial pairs.

Layout per core (4 images, 2 groups of 128 targets on partitions):
  partition p in [0,128), group g in {0,1}:
    image j = g*2 + p//64 (local), target t = p%64, channel c in [0,7)
    tvals[p, g*7+c]  = targets[j, t, c]
    cbase[p, g*7+c]  = j*C*H*W + c*H*W   (flat element base, f32-exact)
  flat gather offset = cbase + (gy*400 + gx), gy/gx = floor(coord * 5.0)
"""

import numpy as np

B, C, H, W = 32, 7, 400, 400
T = 64
NCORES = 8
BLOC = B // NCORES          # 4 images per core
HW = H * W                  # 160000
CHW = C * HW                # 1120000
NELEM = BLOC * CHW          # 4480000 elements per core
NG = BLOC * T // 128        # 2 groups of 128 targets
P = 128
GC = NG * C                 # 14 columns

_cached = {}
TRACE = False  # set True (e.g. from test.py) to capture an NTFF profile


def _build_nc():
    import concourse.bacc as bacc
    import concourse.bass as bass
    import concourse.tile as tile
    import concourse.mybir as mybir

    f32 = mybir.dt.float32
    i32 = mybir.dt.int32
    AF = mybir.ActivationFunctionType
    OP = mybir.AluOpType
    AX = mybir.AxisListType

    nc = bacc.Bacc(
        "TRN2",
        target_bir_lowering=False,
        debug=False,
        enable_asserts=True,
        num_devices=NCORES,
    )

    preds_flat = nc.dram_tensor("preds_flat", [NELEM, 1], f32, kind="ExternalInput")
    tvals = nc.dram_tensor("tvals", [P, GC], f32, kind="ExternalInput")
    t01 = nc.dram_tensor("t01", [P, 2 * NG], f32, kind="ExternalInput")
    jbase = nc.dram_tensor("jbase", [P, NG], f32, kind="ExternalInput")
    ut = nc.dram_tensor("ut", [P, P], f32, kind="ExternalInput")
    id128 = nc.dram_tensor("id128", [P, P], f32, kind="ExternalInput")
    onehalf = nc.dram_tensor("onehalf", [2, 1], f32, kind="ExternalInput")
    out_d = nc.dram_tensor("out", [2, 1], f32, kind="ExternalOutput")

    with tile.TileContext(nc) as tc:
        with (
            tc.tile_pool(name="sbuf", bufs=1) as sb,
            tc.tile_pool(name="psum", bufs=1, space="PSUM") as pp,
            tc.tile_pool(name="dram", bufs=1, space="DRAM") as dp,
        ):
            tv = sb.tile([P, GC], f32)
            nc.sync.dma_start(tv[:], tvals[:, :])
            txy = sb.tile([P, 2 * NG], f32)
            nc.sync.dma_start(txy[:], t01[:, :])
            jb = sb.tile([P, NG], f32)
            nc.sync.dma_start(jb[:], jbase[:, :])
            utt = sb.tile([P, P], f32)
            nc.sync.dma_start(utt[:], ut[:, :])
            idt = sb.tile([P, P], f32)
            nc.sync.dma_start(idt[:], id128[:, :])
            oht = sb.tile([2, 1], f32)
            nc.sync.dma_start(oht[:], onehalf[:, :])

            # grid coords: fl = floor(coord * 5.0).  HW-safe floor: round-trip
            # through int32 (any rounding mode), then subtract 1 where the
            # round-tripped value exceeds x.
            a = sb.tile([P, 2 * NG], f32)
            nc.vector.tensor_scalar_mul(a[:], txy[:], 5.0)
            ci = sb.tile([P, 2 * NG], i32)
            nc.vector.tensor_copy(ci[:], a[:])
            cf = sb.tile([P, 2 * NG], f32)
            nc.vector.tensor_copy(cf[:], ci[:])
            corr = sb.tile([P, 2 * NG], f32)
            nc.vector.tensor_tensor(corr[:], cf[:], a[:], OP.is_gt)
            fl = sb.tile([P, 2 * NG], f32)
            nc.vector.tensor_sub(fl[:], cf[:], corr[:])
            # npos = gy*400 + gx   (cols 0:NG of fl are gx, NG:2NG are gy)
            gy400 = sb.tile([P, NG], f32)
            nc.vector.tensor_scalar_mul(gy400[:], fl[:, NG : 2 * NG], float(W))
            npos = sb.tile([P, NG], f32)
            nc.vector.tensor_add(npos[:], gy400[:], fl[:, 0:NG])

            # flat element offsets into the host-transposed [b,y,x,c] shard:
            # offs = npos*C + j*C*H*W, exact ints < 2^24
            offs_f = sb.tile([P, NG], f32)
            for g in range(NG):
                nc.vector.tensor_scalar(
                    offs_f[:, g : g + 1],
                    npos[:, g : g + 1],
                    float(C),
                    jb[:, g : g + 1],
                    OP.mult,
                    OP.add,
                )
            offs_i = sb.tile([P, NG], i32)
            nc.vector.tensor_copy(offs_i[:], offs_f[:])

            # gather preds at the scattered cells.  HW indirect DMA consumes
            # ONE offset per partition row and fetches a contiguous run, so
            # with channels-last layout one gather per group moves 7 floats
            # per target.
            gat = sb.tile([P, GC], f32)
            for g in range(NG):
                nc.gpsimd.indirect_dma_start(
                    out=gat[:, g * C : (g + 1) * C],
                    out_offset=None,
                    in_=preds_flat[:, :],
                    in_offset=bass.IndirectOffsetOnAxis(
                        ap=offs_i[:, g : g + 1], axis=0
                    ),
                )

            # 2*smoothl1(d) = min(|d|,1) * (|d| + relu(|d|-1))
            d = sb.tile([P, GC], f32)
            nc.vector.tensor_sub(d[:], gat[:], tv[:])
            nd = sb.tile([P, GC], f32)
            nc.vector.tensor_scalar_mul(nd[:], d[:], -1.0)
            ad = sb.tile([P, GC], f32)
            nc.vector.tensor_tensor(ad[:], d[:], nd[:], OP.max)
            mn = sb.tile([P, GC], f32)
            nc.vector.tensor_scalar_min(mn[:], ad[:], 1.0)
            r = sb.tile([P, GC], f32)
            nc.vector.tensor_scalar(r[:], ad[:], 1.0, 0.0, OP.subtract, OP.max)
            s = sb.tile([P, GC], f32)
            nc.vector.tensor_add(s[:], ad[:], r[:])
            le = sb.tile([P, GC], f32)
            nc.vector.tensor_mul(le[:], mn[:], s[:])

            # last-writer-wins winner mask per group; rhs cols = (loss_row, win)
            rhs = sb.tile([P, 2], f32)
            win2 = sb.tile([P, NG], f32)
            lw2 = sb.tile([P, NG], f32)
            for g in range(NG):
                posb = offs_f[:, g : g + 1].to_broadcast([P, P])
                pT_ps = pp.tile([P, P], f32, tag=f"tps{g}")
                nc.tensor.transpose(pT_ps[:], posb, idt[:])
                pT = sb.tile([P, P], f32, tag=f"pT{g}")
                nc.vector.tensor_copy(pT[:], pT_ps[:])
                eq = sb.tile([P, P], f32, tag=f"eq{g}")
                nc.vector.tensor_tensor(eq[:], posb, pT[:], OP.is_equal)
                msk = sb.tile([P, P], f32, tag=f"msk{g}")
                nc.vector.tensor_mul(msk[:], eq[:], utt[:])
                coll = sb.tile([P, 1], f32, tag=f"coll{g}")
                nc.vector.reduce_max(coll[:], msk[:], axis=AX.X)
                nc.vector.tensor_scalar(
                    win2[:, g : g + 1], coll[:], -1.0, 1.0, OP.mult, OP.add
                )
                lrow = sb.tile([P, 1], f32, tag=f"lrow{g}")
                nc.vector.reduce_sum(lrow[:], le[:, g * C : (g + 1) * C], axis=AX.X)
                nc.vector.tensor_mul(
                    lw2[:, g : g + 1], lrow[:], win2[:, g : g + 1]
                )
            nc.vector.tensor_add(rhs[:, 0:1], lw2[:, 0:1], lw2[:, 1:2])
            nc.vector.tensor_add(rhs[:, 1:2], win2[:, 0:1], win2[:, 1:2])

            # exact partition reduction: bounce [P,2] through DRAM, read back
            # transposed as [2,P], reduce on DVE (PE fp32 matmul is fp32r on
            # HW and loses precision).  Scale loss row by 0.5 at the end.
            scratch = dp.tile([P, 2], f32)
            nc.sync.dma_start(scratch[:], rhs[:])
            tr = sb.tile([2, P], f32)
            nc.sync.dma_start(tr[:], scratch[:].rearrange("p c -> c p"))
            red = sb.tile([2, 1], f32)
            nc.vector.reduce_sum(red[:], tr[:], axis=AX.X)
            outt = sb.tile([2, 1], f32)
            nc.vector.tensor_tensor(outt[:], red[:], oht[:], OP.mult)
            nc.sync.dma_start(out_d[:, :], outt[:])

    nc.compile()
    return nc


def _get_nc():
    if "nc" not in _cached:
        _cached["nc"] = _build_nc()
    return _cached["nc"]


def _make_in_maps(preds, targets):
    jj = (np.arange(P) // 64)[:, None]
    gg = np.arange(NG)[None, :]
    jbase = ((gg * 2 + jj) * CHW).astype(np.float32)
    ut = np.triu(np.ones((P, P), np.float32), k=1)
    id128 = np.eye(P, dtype=np.float32)
    onehalf = np.array([[0.5], [1.0]], np.float32)

    # channels-last relayout so each target's 7 channels are one contiguous
    # 28B indirect-DMA row
    preds_t = np.ascontiguousarray(preds.transpose(0, 2, 3, 1))

    in_maps = []
    for k in range(NCORES):
        pshard = preds_t[k * BLOC : (k + 1) * BLOC].reshape(NELEM, 1)
        tshard = targets[k * BLOC : (k + 1) * BLOC]  # [4, 64, 7]
        # tvals[p, g*7+c] = tshard[g*2 + p//64, p%64, c]
        tvals = np.ascontiguousarray(
            tshard.reshape(NG, 2, T, C).transpose(1, 2, 0, 3).reshape(P, GC)
        )
        # t01 cols: [x_g0, x_g1, y_g0, y_g1]
        t01 = np.ascontiguousarray(
            np.stack(
                [tvals[:, 0], tvals[:, C], tvals[:, 1], tvals[:, C + 1]], axis=1
            )
        )
        in_maps.append(
            {
                "preds_flat": pshard,
                "tvals": tvals,
                "t01": t01,
                "jbase": jbase,
                "ut": ut,
                "id128": id128,
                "onehalf": onehalf,
            }
        )
    return in_maps


def kernel(preds, targets):
    from concourse.bass_utils import run_bass_kernel_spmd

    preds = np.ascontiguousarray(np.asarray(preds), dtype=np.float32)
    targets = np.ascontiguousarray(np.asarray(targets), dtype=np.float32)
    assert preds.shape == (B, C, H, W) and targets.shape == (B, T, C)

    nc = _get_nc()
    in_maps = _make_in_maps(preds, targets)
    res = run_bass_kernel_spmd(nc, in_maps, list(range(NCORES)), trace=TRACE)
    _cached["last_results"] = res

    lsum = np.float32(0.0)
    nsum = np.float32(0.0)
    for k in range(NCORES):
        part = res.results[k]["out"].reshape(2)
        lsum = np.float32(lsum + np.float32(part[0]))
        nsum = np.float32(nsum + np.float32(part[1]))
    loss = np.float32(lsum / np.float32(nsum + np.float32(1e-6)))
    return loss, nsum


# revision 19
# speedup vs baseline: 2.2251x; 1.4930x over previous
"""Detection-loss kernel for Trainium2 (8 NeuronCores, data-parallel over batch).

Reference computes: scatter 64 targets/image into a [B,C,H,W] map + mask,
then masked SmoothL1(preds, map).sum() / num_objects.

Key observation: the mask is nonzero at <= B*T positions, so the loss only
depends on preds at those positions.  Instead of streaming the 143MB preds
tensor, each core *gathers* preds at its images' (gy,gx) cells via indirect
DMA (1792 elements/core), resolves duplicate-cell collisions with
last-writer-wins (matching jax scatter semantics), and reduces two partial
scalars.  Host combines the 8 partial pairs.

Sharding layout per core (4 images, 2 groups of 128 targets on partitions):
  partition p in [0,128), group g in {0,1}:
    image j = g*2 + p//64 (local), target t = p%64, channel c in [0,7)
  preds are host-relayouted channels-last ([b,y,x,c]) so one indirect-DMA
  descriptor per target moves all 7 channels (28B contiguous).
  flat gather offset = (gy*W + gx)*C + j*C*H*W, gy/gx = floor(coord * 5.0).
"""

import numpy as np

B, C, H, W = 32, 7, 400, 400
T = 64
NCORES = 8
BLOC = B // NCORES          # 4 images per core
HW = H * W                  # 160000
CHW = C * HW                # 1120000
NELEM = BLOC * CHW          # 4480000 elements per core
NG = BLOC * T // 128        # 2 groups of 128 targets
P = 128
GC = NG * C                 # 14 value columns
BIGM = float(2**25)         # collision-mask offset (kills eq below diagonal)

_cached = {}
TRACE = False  # set True (e.g. from test.py) to capture an NTFF profile


def _build_nc():
    import concourse.bacc as bacc
    import concourse.bass as bass
    import concourse.tile as tile
    import concourse.mybir as mybir

    f32 = mybir.dt.float32
    i32 = mybir.dt.int32
    OP = mybir.AluOpType
    AX = mybir.AxisListType

    nc = bacc.Bacc(
        "TRN2",
        target_bir_lowering=False,
        debug=False,
        enable_asserts=True,
        num_devices=NCORES,
    )

    preds_flat = nc.dram_tensor("preds_flat", [NELEM, 1], f32, kind="ExternalInput")
    # aux1: [t01 (4) | jbase (2)] — the small operands the coord chain needs
    aux1 = nc.dram_tensor("aux1", [P, 6], f32, kind="ExternalInput")
    # aux2: [tvals (14) | utm (128) | id128 (128)]
    aux2 = nc.dram_tensor("aux2", [P, GC + 2 * P], f32, kind="ExternalInput")
    out_d = nc.dram_tensor("out", [2, 1], f32, kind="ExternalOutput")

    with tile.TileContext(nc) as tc:
        with (
            tc.tile_pool(name="sbuf", bufs=1) as sb,
            tc.tile_pool(name="psum", bufs=1, space="PSUM") as pp,
        ):
            x1 = sb.tile([P, 6], f32)
            nc.sync.dma_start(x1[:], aux1[:, :])
            x2 = sb.tile([P, GC + 2 * P], f32)
            nc.sync.dma_start(x2[:], aux2[:, :])
            tv = x2[:, 0:GC]
            utm = x2[:, GC : GC + P]
            idt = x2[:, GC + P : GC + 2 * P]

            # grid coords: floor(coord*5) via int32 round-trip (any rounding
            # mode) corrected where the round-trip exceeded the input
            a = sb.tile([P, 2 * NG], f32)
            nc.vector.tensor_scalar_mul(a[:], x1[:, 0 : 2 * NG], 5.0)
            ci = sb.tile([P, 2 * NG], i32)
            nc.vector.tensor_copy(ci[:], a[:])
            cf = sb.tile([P, 2 * NG], f32)
            nc.vector.tensor_copy(cf[:], ci[:])
            corr = sb.tile([P, 2 * NG], f32)
            nc.vector.tensor_tensor(corr[:], cf[:], a[:], OP.is_gt)
            fl = sb.tile([P, 2 * NG], f32)
            nc.vector.tensor_sub(fl[:], cf[:], corr[:])
            # npos = gy*W + gx ; offs = npos*C + j*C*H*W   (exact ints < 2^23)
            npos = sb.tile([P, NG], f32)
            nc.vector.scalar_tensor_tensor(
                npos[:], fl[:, NG : 2 * NG], float(W), fl[:, 0:NG], OP.mult, OP.add
            )
            offs_f = sb.tile([P, NG], f32)
            nc.vector.scalar_tensor_tensor(
                offs_f[:], npos[:], float(C), x1[:, 4:6], OP.mult, OP.add
            )
            offs_i = sb.tile([P, NG], i32)
            nc.vector.tensor_copy(offs_i[:], offs_f[:])

            # gather: one 28B descriptor per target (channels-last layout)
            gat = sb.tile([P, GC], f32)
            for g in range(NG):
                nc.gpsimd.indirect_dma_start(
                    out=gat[:, g * C : (g + 1) * C],
                    out_offset=None,
                    in_=preds_flat[:, :],
                    in_offset=bass.IndirectOffsetOnAxis(
                        ap=offs_i[:, g : g + 1], axis=0
                    ),
                )

            # last-writer-wins winner mask per group (overlaps the gather):
            # pT[p,q] = pos[q] (PE transpose, bit-exact); +BIGM on/below the
            # diagonal makes eq impossible there, so a row-max of equality
            # flags collisions with a LATER target.
            win2 = sb.tile([P, NG], f32)
            for g in range(NG):
                posb = offs_f[:, g : g + 1].to_broadcast([P, P])
                pT_ps = pp.tile([P, P], f32, tag=f"tps{g}")
                nc.tensor.transpose(pT_ps[:], posb, idt[:])
                pTm = sb.tile([P, P], f32, tag=f"pTm{g}")
                nc.vector.tensor_add(pTm[:], pT_ps[:], utm[:])
                eq = sb.tile([P, P], f32, tag=f"eq{g}")
                nc.vector.tensor_tensor(eq[:], posb, pTm[:], OP.is_equal)
                coll = sb.tile([P, 1], f32, tag=f"coll{g}")
                nc.vector.reduce_max(coll[:], eq[:], axis=AX.X)
                nc.vector.tensor_scalar(
                    win2[:, g : g + 1], coll[:], -1.0, 1.0, OP.mult, OP.add
                )

            # smoothl1(d) = (0.5*min(|d|,1)) * (|d| + relu(|d|-1))
            d = sb.tile([P, GC], f32)
            nc.vector.tensor_sub(d[:], gat[:], tv[:])
            ad = sb.tile([P, GC], f32)
            nc.vector.scalar_tensor_tensor(ad[:], d[:], -1.0, d[:], OP.mult, OP.max)
            mnh = sb.tile([P, GC], f32)
            nc.vector.tensor_scalar(mnh[:], ad[:], 1.0, 0.5, OP.min, OP.mult)
            r = sb.tile([P, GC], f32)
            nc.vector.tensor_scalar(r[:], ad[:], 1.0, 0.0, OP.subtract, OP.max)
            s = sb.tile([P, GC], f32)
            nc.vector.tensor_add(s[:], ad[:], r[:])
            le = sb.tile([P, GC], f32)
            nc.vector.tensor_mul(le[:], mnh[:], s[:])

            # per-target row sums (both groups in one reduce), apply winner
            lrow2 = sb.tile([P, NG], f32)
            nc.vector.reduce_sum(
                lrow2[:], le[:].rearrange("p (g c) -> p g c", c=C), axis=AX.X
            )
            lw2 = sb.tile([P, NG], f32)
            nc.vector.tensor_mul(lw2[:], lrow2[:], win2[:])
            rhs = sb.tile([P, 2], f32)
            nc.vector.tensor_add(rhs[:, 0:1], lw2[:, 0:1], lw2[:, 1:2])
            nc.vector.tensor_add(rhs[:, 1:2], win2[:, 0:1], win2[:, 1:2])

            # exact partition reduction: PE transpose (bit-exact move) then
            # DVE reduce straight out of PSUM
            tps = pp.tile([2, P], f32, tag="tfin")
            nc.tensor.transpose(tps[:], rhs[:], idt[:])
            red = sb.tile([2, 1], f32)
            nc.vector.reduce_sum(red[:], tps[:], axis=AX.X)
            nc.sync.dma_start(out_d[:, :], red[:])

    nc.compile()
    return nc


def _get_nc():
    if "nc" not in _cached:
        _cached["nc"] = _build_nc()
    return _cached["nc"]


def _make_in_maps(preds, targets):
    jj = (np.arange(P) // 64)[:, None]
    gg = np.arange(NG)[None, :]
    jbase = ((gg * 2 + jj) * CHW).astype(np.float32)
    rr = np.arange(P)
    utm = np.where(rr[None, :] > rr[:, None], 0.0, BIGM).astype(np.float32)
    id128 = np.eye(P, dtype=np.float32)

    # channels-last relayout so each target's 7 channels are one contiguous
    # 28B indirect-DMA row
    preds_t = np.ascontiguousarray(preds.transpose(0, 2, 3, 1))

    in_maps = []
    for k in range(NCORES):
        pshard = preds_t[k * BLOC : (k + 1) * BLOC].reshape(NELEM, 1)
        tshard = targets[k * BLOC : (k + 1) * BLOC]  # [4, 64, 7]
        # tvals[p, g*7+c] = tshard[g*2 + p//64, p%64, c]
        tvals = tshard.reshape(NG, 2, T, C).transpose(1, 2, 0, 3).reshape(P, GC)
        # t01 cols: [x_g0, x_g1, y_g0, y_g1]
        t01 = np.stack(
            [tvals[:, 0], tvals[:, C], tvals[:, 1], tvals[:, C + 1]], axis=1
        )
        aux1 = np.ascontiguousarray(np.hstack([t01, jbase]).astype(np.float32))
        aux2 = np.ascontiguousarray(
            np.hstack([tvals, utm, id128]).astype(np.float32)
        )
        in_maps.append({"preds_flat": pshard, "aux1": aux1, "aux2": aux2})
    return in_maps


def kernel(preds, targets):
    from concourse.bass_utils import run_bass_kernel_spmd

    preds = np.ascontiguousarray(np.asarray(preds), dtype=np.float32)
    targets = np.ascontiguousarray(np.asarray(targets), dtype=np.float32)
    assert preds.shape == (B, C, H, W) and targets.shape == (B, T, C)

    nc = _get_nc()
    in_maps = _make_in_maps(preds, targets)
    res = run_bass_kernel_spmd(nc, in_maps, list(range(NCORES)), trace=TRACE)
    _cached["last_results"] = res

    lsum = np.float32(0.0)
    nsum = np.float32(0.0)
    for k in range(NCORES):
        part = res.results[k]["out"].reshape(2)
        lsum = np.float32(lsum + np.float32(part[0]))
        nsum = np.float32(nsum + np.float32(part[1]))
    loss = np.float32(lsum / np.float32(nsum + np.float32(1e-6)))
    return loss, nsum


# revision 20
# speedup vs baseline: 2.2261x; 1.0005x over previous
"""Detection-loss kernel for Trainium2 (8 NeuronCores, data-parallel over batch).

Reference computes: scatter 64 targets/image into a [B,C,H,W] map + mask,
then masked SmoothL1(preds, map).sum() / num_objects.

Key observation: the mask is nonzero at <= B*T positions, so the loss only
depends on preds at those positions.  Instead of streaming the 143MB preds
tensor, each core *gathers* preds at its images' (gy,gx) cells via indirect
DMA (1792 elements/core), resolves duplicate-cell collisions with
last-writer-wins (matching jax scatter semantics), and reduces two partial
scalars.  Host combines the 8 partial pairs.

Sharding layout per core (4 images, 2 groups of 128 targets on partitions):
  partition p in [0,128), group g in {0,1}:
    image j = g*2 + p//64 (local), target t = p%64, channel c in [0,7)
  preds are host-relayouted channels-last ([b,y,x,c]) so one indirect-DMA
  descriptor per target moves all 7 channels (28B contiguous).
  flat gather offset = (gy*W + gx)*C + j*C*H*W, gy/gx = floor(coord * 5.0).
"""

import numpy as np

B, C, H, W = 32, 7, 400, 400
T = 64
NCORES = 8
BLOC = B // NCORES          # 4 images per core
HW = H * W                  # 160000
CHW = C * HW                # 1120000
NELEM = BLOC * CHW          # 4480000 elements per core
NG = BLOC * T // 128        # 2 groups of 128 targets
P = 128
GC = NG * C                 # 14 value columns
BIGM = float(2**25)         # collision-mask offset (kills eq below diagonal)

_cached = {}
TRACE = False  # set True (e.g. from test.py) to capture an NTFF profile


def _build_nc():
    import concourse.bacc as bacc
    import concourse.bass as bass
    import concourse.tile as tile
    import concourse.mybir as mybir

    f32 = mybir.dt.float32
    i32 = mybir.dt.int32
    OP = mybir.AluOpType
    AX = mybir.AxisListType

    nc = bacc.Bacc(
        "TRN2",
        target_bir_lowering=False,
        debug=False,
        enable_asserts=False,
        num_devices=NCORES,
    )

    preds_flat = nc.dram_tensor("preds_flat", [NELEM, 1], f32, kind="ExternalInput")
    # aux1: [t01 (4) | jbase (2)] — the small operands the coord chain needs
    aux1 = nc.dram_tensor("aux1", [P, 6], f32, kind="ExternalInput")
    # aux2: [tvals (14) | utm (128) | id128 (128)]
    aux2 = nc.dram_tensor("aux2", [P, GC + 2 * P], f32, kind="ExternalInput")
    out_d = nc.dram_tensor("out", [2, 1], f32, kind="ExternalOutput")

    with tile.TileContext(nc) as tc:
        with (
            tc.tile_pool(name="sbuf", bufs=1) as sb,
            tc.tile_pool(name="psum", bufs=1, space="PSUM") as pp,
        ):
            x1 = sb.tile([P, 6], f32)
            nc.sync.dma_start(x1[:], aux1[:, :])
            x2 = sb.tile([P, GC + 2 * P], f32)
            nc.sync.dma_start(x2[:], aux2[:, :])
            tv = x2[:, 0:GC]
            utm = x2[:, GC : GC + P]
            idt = x2[:, GC + P : GC + 2 * P]

            # grid coords: floor(coord*5) via int32 round-trip (any rounding
            # mode) corrected where the round-trip exceeded the input
            a = sb.tile([P, 2 * NG], f32)
            nc.vector.tensor_scalar_mul(a[:], x1[:, 0 : 2 * NG], 5.0)
            ci = sb.tile([P, 2 * NG], i32)
            nc.vector.tensor_copy(ci[:], a[:])
            cf = sb.tile([P, 2 * NG], f32)
            nc.vector.tensor_copy(cf[:], ci[:])
            corr = sb.tile([P, 2 * NG], f32)
            nc.vector.tensor_tensor(corr[:], cf[:], a[:], OP.is_gt)
            fl = sb.tile([P, 2 * NG], f32)
            nc.vector.tensor_sub(fl[:], cf[:], corr[:])
            # npos = gy*W + gx ; offs = npos*C + j*C*H*W   (exact ints < 2^23)
            npos = sb.tile([P, NG], f32)
            nc.vector.scalar_tensor_tensor(
                npos[:], fl[:, NG : 2 * NG], float(W), fl[:, 0:NG], OP.mult, OP.add
            )
            offs_f = sb.tile([P, NG], f32)
            nc.vector.scalar_tensor_tensor(
                offs_f[:], npos[:], float(C), x1[:, 4:6], OP.mult, OP.add
            )
            offs_i = sb.tile([P, NG], i32)
            nc.vector.tensor_copy(offs_i[:], offs_f[:])

            # gather: one 28B descriptor per target (channels-last layout)
            gat = sb.tile([P, GC], f32)
            for g in range(NG):
                nc.gpsimd.indirect_dma_start(
                    out=gat[:, g * C : (g + 1) * C],
                    out_offset=None,
                    in_=preds_flat[:, :],
                    in_offset=bass.IndirectOffsetOnAxis(
                        ap=offs_i[:, g : g + 1], axis=0
                    ),
                )

            # last-writer-wins winner mask per group (overlaps the gather):
            # pT[p,q] = pos[q] (PE transpose, bit-exact); +BIGM on/below the
            # diagonal makes eq impossible there, so a row-max of equality
            # flags collisions with a LATER target.
            win2 = sb.tile([P, NG], f32)
            for g in range(NG):
                posb = offs_f[:, g : g + 1].to_broadcast([P, P])
                pT_ps = pp.tile([P, P], f32, tag=f"tps{g}")
                nc.tensor.transpose(pT_ps[:], posb, idt[:])
                pTm = sb.tile([P, P], f32, tag=f"pTm{g}")
                nc.vector.tensor_add(pTm[:], pT_ps[:], utm[:])
                eq = sb.tile([P, P], f32, tag=f"eq{g}")
                nc.vector.tensor_tensor(eq[:], posb, pTm[:], OP.is_equal)
                coll = sb.tile([P, 1], f32, tag=f"coll{g}")
                nc.vector.reduce_max(coll[:], eq[:], axis=AX.X)
                nc.vector.tensor_scalar(
                    win2[:, g : g + 1], coll[:], -1.0, 1.0, OP.mult, OP.add
                )

            # smoothl1(d) = (0.5*min(|d|,1)) * (|d| + relu(|d|-1))
            d = sb.tile([P, GC], f32)
            nc.vector.tensor_sub(d[:], gat[:], tv[:])
            ad = sb.tile([P, GC], f32)
            nc.vector.scalar_tensor_tensor(ad[:], d[:], -1.0, d[:], OP.mult, OP.max)
            mnh = sb.tile([P, GC], f32)
            nc.vector.tensor_scalar(mnh[:], ad[:], 1.0, 0.5, OP.min, OP.mult)
            r = sb.tile([P, GC], f32)
            nc.vector.tensor_scalar(r[:], ad[:], 1.0, 0.0, OP.subtract, OP.max)
            s = sb.tile([P, GC], f32)
            nc.vector.tensor_add(s[:], ad[:], r[:])
            le = sb.tile([P, GC], f32)
            nc.vector.tensor_mul(le[:], mnh[:], s[:])

            # per-target row sums (both groups in one reduce), apply winner
            lrow2 = sb.tile([P, NG], f32)
            nc.vector.reduce_sum(
                lrow2[:], le[:].rearrange("p (g c) -> p g c", c=C), axis=AX.X
            )
            lw2 = sb.tile([P, NG], f32)
            nc.vector.tensor_mul(lw2[:], lrow2[:], win2[:])
            rhs = sb.tile([P, 2], f32)
            nc.vector.tensor_add(rhs[:, 0:1], lw2[:, 0:1], lw2[:, 1:2])
            nc.vector.tensor_add(rhs[:, 1:2], win2[:, 0:1], win2[:, 1:2])

            # exact partition reduction: PE transpose (bit-exact move) then
            # DVE reduce straight out of PSUM
            tps = pp.tile([2, P], f32, tag="tfin")
            nc.tensor.transpose(tps[:], rhs[:], idt[:])
            red = sb.tile([2, 1], f32)
            nc.vector.reduce_sum(red[:], tps[:], axis=AX.X)
            nc.sync.dma_start(out_d[:, :], red[:])

    nc.compile()
    return nc


def _get_nc():
    if "nc" not in _cached:
        _cached["nc"] = _build_nc()
    return _cached["nc"]


def _make_in_maps(preds, targets):
    jj = (np.arange(P) // 64)[:, None]
    gg = np.arange(NG)[None, :]
    jbase = ((gg * 2 + jj) * CHW).astype(np.float32)
    rr = np.arange(P)
    utm = np.where(rr[None, :] > rr[:, None], 0.0, BIGM).astype(np.float32)
    id128 = np.eye(P, dtype=np.float32)

    # channels-last relayout so each target's 7 channels are one contiguous
    # 28B indirect-DMA row
    preds_t = np.ascontiguousarray(preds.transpose(0, 2, 3, 1))

    in_maps = []
    for k in range(NCORES):
        pshard = preds_t[k * BLOC : (k + 1) * BLOC].reshape(NELEM, 1)
        tshard = targets[k * BLOC : (k + 1) * BLOC]  # [4, 64, 7]
        # tvals[p, g*7+c] = tshard[g*2 + p//64, p%64, c]
        tvals = tshard.reshape(NG, 2, T, C).transpose(1, 2, 0, 3).reshape(P, GC)
        # t01 cols: [x_g0, x_g1, y_g0, y_g1]
        t01 = np.stack(
            [tvals[:, 0], tvals[:, C], tvals[:, 1], tvals[:, C + 1]], axis=1
        )
        aux1 = np.ascontiguousarray(np.hstack([t01, jbase]).astype(np.float32))
        aux2 = np.ascontiguousarray(
            np.hstack([tvals, utm, id128]).astype(np.float32)
        )
        in_maps.append({"preds_flat": pshard, "aux1": aux1, "aux2": aux2})
    return in_maps


def kernel(preds, targets):
    from concourse.bass_utils import run_bass_kernel_spmd

    preds = np.ascontiguousarray(np.asarray(preds), dtype=np.float32)
    targets = np.ascontiguousarray(np.asarray(targets), dtype=np.float32)
    assert preds.shape == (B, C, H, W) and targets.shape == (B, T, C)

    nc = _get_nc()
    in_maps = _make_in_maps(preds, targets)
    res = run_bass_kernel_spmd(nc, in_maps, list(range(NCORES)), trace=TRACE)
    _cached["last_results"] = res

    lsum = np.float32(0.0)
    nsum = np.float32(0.0)
    for k in range(NCORES):
        part = res.results[k]["out"].reshape(2)
        lsum = np.float32(lsum + np.float32(part[0]))
        nsum = np.float32(nsum + np.float32(part[1]))
    loss = np.float32(lsum / np.float32(nsum + np.float32(1e-6)))
    return loss, nsum


# revision 22
# speedup vs baseline: 2.2428x; 1.0075x over previous
"""Detection-loss kernel for Trainium2 (8 NeuronCores, data-parallel over batch).

Reference computes: scatter 64 targets/image into a [B,C,H,W] map + mask,
then masked SmoothL1(preds, map).sum() / num_objects.

Key observation: the mask is nonzero at <= B*T positions, so the loss only
depends on preds at those positions.  Instead of streaming the 143MB preds
tensor, each core *gathers* preds at its images' (gy,gx) cells via indirect
DMA (1792 elements/core), resolves duplicate-cell collisions with
last-writer-wins (matching jax scatter semantics), and reduces two partial
scalars.  Host combines the 8 partial pairs.

Sharding layout per core (4 images, 2 groups of 128 targets on partitions):
  partition p in [0,128), group g in {0,1}:
    image j = g*2 + p//64 (local), target t = p%64, channel c in [0,7)
  preds are host-relayouted channels-last ([b,y,x,c]) so one indirect-DMA
  descriptor per target moves all 7 channels (28B contiguous).
  flat gather offset = (gy*W + gx)*C + j*C*H*W, gy/gx = floor(coord * 5.0).
"""

import numpy as np

B, C, H, W = 32, 7, 400, 400
T = 64
NCORES = 8
BLOC = B // NCORES          # 4 images per core
HW = H * W                  # 160000
CHW = C * HW                # 1120000
NELEM = BLOC * CHW          # 4480000 elements per core
NG = BLOC * T // 128        # 2 groups of 128 targets
P = 128
GC = NG * C                 # 14 value columns
BIGM = float(2**25)         # collision-mask offset (kills eq below diagonal)

_cached = {}
TRACE = False  # set True (e.g. from test.py) to capture an NTFF profile


def _build_nc():
    import concourse.bacc as bacc
    import concourse.bass as bass
    import concourse.tile as tile
    import concourse.mybir as mybir

    f32 = mybir.dt.float32
    i32 = mybir.dt.int32
    OP = mybir.AluOpType
    AX = mybir.AxisListType

    nc = bacc.Bacc(
        "TRN2",
        target_bir_lowering=False,
        debug=False,
        enable_asserts=False,
        num_devices=NCORES,
    )

    preds_flat = nc.dram_tensor("preds_flat", [NELEM, 1], f32, kind="ExternalInput")
    # aux1: [t01 (4) | jbase (2)] — the small operands the coord chain needs
    aux1 = nc.dram_tensor("aux1", [P, 6], f32, kind="ExternalInput")
    # aux2: [tvals (14) | utm (128) | id128 (128)]
    aux2 = nc.dram_tensor("aux2", [P, GC + 2 * P], f32, kind="ExternalInput")
    out_d = nc.dram_tensor("out", [2, 1], f32, kind="ExternalOutput")

    with tile.TileContext(nc) as tc:
        with (
            tc.tile_pool(name="sbuf", bufs=1) as sb,
            tc.tile_pool(name="psum", bufs=1, space="PSUM") as pp,
        ):
            x1 = sb.tile([P, 6], f32)
            nc.sync.dma_start(x1[:], aux1[:, :])
            x2 = sb.tile([P, GC + 2 * P], f32)
            nc.sync.dma_start(x2[:], aux2[:, :])
            tv = x2[:, 0:GC]
            utm = x2[:, GC : GC + P]
            idt = x2[:, GC + P : GC + 2 * P]

            # grid coords: floor(coord*5) via int32 round-trip (any rounding
            # mode) corrected where the round-trip exceeded the input
            a = sb.tile([P, 2 * NG], f32)
            nc.vector.tensor_scalar_mul(a[:], x1[:, 0 : 2 * NG], 5.0)
            ci = sb.tile([P, 2 * NG], i32)
            nc.vector.tensor_copy(ci[:], a[:])
            cf = sb.tile([P, 2 * NG], f32)
            nc.vector.tensor_copy(cf[:], ci[:])
            corr = sb.tile([P, 2 * NG], f32)
            nc.vector.tensor_tensor(corr[:], cf[:], a[:], OP.is_gt)
            fl = sb.tile([P, 2 * NG], f32)
            nc.vector.tensor_sub(fl[:], cf[:], corr[:])
            # npos = gy*W + gx ; offs = npos*C + j*C*H*W   (exact ints < 2^23)
            npos = sb.tile([P, NG], f32)
            nc.vector.scalar_tensor_tensor(
                npos[:], fl[:, NG : 2 * NG], float(W), fl[:, 0:NG], OP.mult, OP.add
            )
            offs_f = sb.tile([P, NG], f32)
            nc.vector.scalar_tensor_tensor(
                offs_f[:], npos[:], float(C), x1[:, 4:6], OP.mult, OP.add
            )
            offs_i = sb.tile([P, NG], i32)
            nc.vector.tensor_copy(offs_i[:], offs_f[:])

            # gather: one 28B descriptor per target (channels-last layout)
            gat = sb.tile([P, GC], f32)
            for g in range(NG):
                nc.gpsimd.indirect_dma_start(
                    out=gat[:, g * C : (g + 1) * C],
                    out_offset=None,
                    in_=preds_flat[:, :],
                    in_offset=bass.IndirectOffsetOnAxis(
                        ap=offs_i[:, g : g + 1], axis=0
                    ),
                )

            # last-writer-wins winner mask per group (overlaps the gather):
            # pT[p,q] = pos[q] (PE transpose, bit-exact); +BIGM on/below the
            # diagonal makes eq impossible there, so a row-max of equality
            # flags collisions with a LATER target.
            win2 = sb.tile([P, NG], f32)
            for g in range(NG):
                posb = offs_f[:, g : g + 1].to_broadcast([P, P])
                pT_ps = pp.tile([P, P], f32, tag=f"tps{g}")
                nc.tensor.transpose(pT_ps[:], posb, idt[:])
                pTm = sb.tile([P, P], f32, tag=f"pTm{g}")
                nc.vector.tensor_add(pTm[:], pT_ps[:], utm[:])
                eq = sb.tile([P, P], f32, tag=f"eq{g}")
                nc.vector.tensor_tensor(eq[:], posb, pTm[:], OP.is_equal)
                coll = sb.tile([P, 1], f32, tag=f"coll{g}")
                nc.vector.reduce_max(coll[:], eq[:], axis=AX.X)
                nc.vector.tensor_scalar(
                    win2[:, g : g + 1], coll[:], -1.0, 1.0, OP.mult, OP.add
                )

            # smoothl1(d) = (0.5*min(|d|,1)) * (|d| + relu(|d|-1))
            d = sb.tile([P, GC], f32)
            nc.vector.tensor_sub(d[:], gat[:], tv[:])
            ad = sb.tile([P, GC], f32)
            nc.vector.scalar_tensor_tensor(ad[:], d[:], -1.0, d[:], OP.mult, OP.max)
            mnh = sb.tile([P, GC], f32)
            nc.vector.tensor_scalar(mnh[:], ad[:], 1.0, 0.5, OP.min, OP.mult)
            r = sb.tile([P, GC], f32)
            nc.vector.tensor_scalar(r[:], ad[:], 1.0, 0.0, OP.subtract, OP.max)
            s = sb.tile([P, GC], f32)
            nc.vector.tensor_add(s[:], ad[:], r[:])
            le = sb.tile([P, GC], f32)
            nc.vector.tensor_mul(le[:], mnh[:], s[:])

            # per-target row sums (both groups in one reduce), apply winner
            lrow2 = sb.tile([P, NG], f32)
            nc.vector.reduce_sum(
                lrow2[:], le[:].rearrange("p (g c) -> p g c", c=C), axis=AX.X
            )
            lw2 = sb.tile([P, NG], f32)
            nc.vector.tensor_mul(lw2[:], lrow2[:], win2[:])
            rhs = sb.tile([P, 2], f32)
            nc.vector.tensor_add(rhs[:, 0:1], lw2[:, 0:1], lw2[:, 1:2])
            nc.vector.tensor_add(rhs[:, 1:2], win2[:, 0:1], win2[:, 1:2])

            # exact partition reduction: PE transpose (bit-exact move) then
            # DVE reduce straight out of PSUM
            tps = pp.tile([2, P], f32, tag="tfin")
            nc.tensor.transpose(tps[:], rhs[:], idt[:])
            red = sb.tile([2, 1], f32)
            nc.vector.reduce_sum(red[:], tps[:], axis=AX.X)
            nc.sync.dma_start(out_d[:, :], red[:])

    nc.compile()
    return nc


def _get_nc():
    if "nc" not in _cached:
        _cached["nc"] = _build_nc()
    return _cached["nc"]


def _make_in_maps(preds, targets):
    jj = (np.arange(P) // 64)[:, None]
    gg = np.arange(NG)[None, :]
    jbase = ((gg * 2 + jj) * CHW).astype(np.float32)
    rr = np.arange(P)
    utm = np.where(rr[None, :] > rr[:, None], 0.0, BIGM).astype(np.float32)
    id128 = np.eye(P, dtype=np.float32)

    # channels-last relayout so each target's 7 channels are one contiguous
    # 28B indirect-DMA row
    preds_t = np.ascontiguousarray(preds.transpose(0, 2, 3, 1))

    in_maps = []
    for k in range(NCORES):
        pshard = preds_t[k * BLOC : (k + 1) * BLOC].reshape(NELEM, 1)
        tshard = targets[k * BLOC : (k + 1) * BLOC]  # [4, 64, 7]
        # tvals[p, g*7+c] = tshard[g*2 + p//64, p%64, c]
        tvals = tshard.reshape(NG, 2, T, C).transpose(1, 2, 0, 3).reshape(P, GC)
        # t01 cols: [x_g0, x_g1, y_g0, y_g1]
        t01 = np.stack(
            [tvals[:, 0], tvals[:, C], tvals[:, 1], tvals[:, C + 1]], axis=1
        )
        aux1 = np.ascontiguousarray(np.hstack([t01, jbase]).astype(np.float32))
        aux2 = np.ascontiguousarray(
            np.hstack([tvals, utm, id128]).astype(np.float32)
        )
        in_maps.append({"preds_flat": pshard, "aux1": aux1, "aux2": aux2})
    return in_maps


def kernel(preds, targets):
    from concourse.bass_utils import run_bass_kernel_spmd

    preds = np.ascontiguousarray(np.asarray(preds), dtype=np.float32)
    targets = np.ascontiguousarray(np.asarray(targets), dtype=np.float32)
    assert preds.shape == (B, C, H, W) and targets.shape == (B, T, C)

    nc = _get_nc()
    in_maps = _make_in_maps(preds, targets)
    res = run_bass_kernel_spmd(nc, in_maps, list(range(NCORES)), trace=TRACE)
    _cached["last_results"] = res

    lsum = np.float32(0.0)
    nsum = np.float32(0.0)
    for k in range(NCORES):
        part = res.results[k]["out"].reshape(2)
        lsum = np.float32(lsum + np.float32(part[0]))
        nsum = np.float32(nsum + np.float32(part[1]))
    loss = np.float32(lsum / np.float32(nsum + np.float32(1e-6)))
    return loss, nsum
